# revision 1
# baseline (speedup 1.0000x reference)
"""DiagonalSSMLayer Trainium2 kernel, v2: fp16 I/O + LN folded into matmul.

Full (unsharded) inputs in, full output out. Data-parallel over batch across
8 NeuronCores (B=8, one batch element per core). Host casts x to fp16
(rel-err budget 2e-2 >> fp16's 5e-4); device loads/stores fp16 (halves HBM
traffic vs f32 -> ~90us DMA floor/core).

Per-core math for x [S=8192, D=1024]:
    mu, var per row (LN stats)
    logits = r * (W_cat @ xT - mu x Wsum) + b      (r = rsqrt(var+eps))
    alpha = sigmoid(logits[0:32]); b = logits[32:64]
    h_t = alpha_t * h_{t-1} + b_t                  (scan along seq)
    out = x + [h;1].T @ [W_out.T; b_out]

Device structure per 512-seq superchunk (16 of them):
  - load x [128, 4, 1024] fp16 (sync HWDGE ring)
  - LN stats: DVE tensor_scalar+accum (S1) + tensor_tensor_reduce (sumsq)
    or bn_stats, per cfg; Pool computes r=rsqrt(var+eps) via Newton + -mu,
    packs into rm8 fp16 [128, 8]
  - PE transposes rm8 -> rmT [8, 128] (row-land stats) and x blocks ->
    xT fp16 in PSUM (is_transpose keeps fp16 => 2x-rate DVE copies out)
  - in-proj: G[64, 512] = W_cat @ xT (PSUM accum) + rank-1 mean correction
    (Wsum x (-mu)) via K=1 matmuls; r_bcast[64, 512] = ones x r via K=1
    matmuls into the same PSUM bank's lower partitions
  - logits = G * r_bcast (DVE); alpha = ACT sigmoid(+b_a); bv = ACT id(+b_in)
  - DVE tensor_tensor_scan -> h fp16, chained via previous chunk's last col
  - out-proj per block: O[128, 1024] f32 = [h;1].T @ [W_out.T; b_out]
  - residual o = O + x -> fp16: DVE fused TT or ACT copy + Pool add (cfg)
  - store o [128, 4, 1024] fp16 (scalar HWDGE ring)
"""

import sys
from contextlib import ExitStack

if "/opt/trn_rl_repo" not in sys.path:
    sys.path.insert(0, "/opt/trn_rl_repo")

import numpy as np

import concourse.bass as bass
import concourse.bacc as bacc
import concourse.tile as tile
from concourse import mybir
from concourse.bass_utils import run_bass_kernel_spmd

F32 = mybir.dt.float32
F16 = mybir.dt.float16
I32 = mybir.dt.int32
OP = mybir.AluOpType
AF = mybir.ActivationFunctionType

B, S, D = 8, 8192, 1024
HN = 32          # H * n state channels
K2 = 2 * HN      # alpha + b fused projection output channels
LN_EPS = 1e-5
RSQRT_MAGIC = 0x5F3759DF

SC = 512         # seq superchunk
NSC = S // SC    # 16
NB = SC // 128   # 4 seq blocks of 128 per superchunk
ND = D // 128    # 8 d-slices

_PROGRAM_CACHE = {}

# engine assignment config
CFG = dict(
    stats="bn",               # "bn": bn_stats/bn_aggr on DVE (the accum_out
                              # variants crash walrus or the device)
    p2=("act", "act", "act", "act"),   # xT PSUM->SBUF copy engine per block
    p3=("dve", "ap", "ap", "dve"),     # residual: "dve" fused | "ap" ACT+Pool
)


def build_program(cfg=CFG, repeat=1):
    nc = bacc.Bacc("TRN2", target_bir_lowering=False, debug=False, num_devices=B)

    x_in = nc.declare_dram_parameter("x", [S, D], F16, isOutput=False)
    w_in_d = nc.declare_dram_parameter("w_in", [128, ND, K2], F16, isOutput=False)
    # Wsum / ones replicated at partitions 0/32/64/96 so per-block K=1
    # rank-1 matmuls can read lhsT and rhs at the same base partition.
    wsum_d = nc.declare_dram_parameter("wsum", [128, K2], F16, isOutput=False)
    ones64_d = nc.declare_dram_parameter("ones64", [128, K2], F16, isOutput=False)
    b_t_d = nc.declare_dram_parameter("b_t", [K2, 1], F32, isOutput=False)
    w_out_d = nc.declare_dram_parameter("w_out", [HN + 1, D], F16, isOutput=False)
    ident_d = nc.declare_dram_parameter("ident", [128, 128], F16, isOutput=False)
    out_d = nc.declare_dram_parameter("out", [S, D], F16, isOutput=True)

    with tile.TileContext(nc) as tc, ExitStack() as ctx:
        consts = ctx.enter_context(tc.tile_pool(name="consts", bufs=1))
        xpool = ctx.enter_context(tc.tile_pool(name="xpool", bufs=3))
        ytpool = ctx.enter_context(tc.tile_pool(name="ytpool", bufs=2))
        stat = ctx.enter_context(tc.tile_pool(name="stat", bufs=3))
        abpool = ctx.enter_context(tc.tile_pool(name="abpool", bufs=2))
        hpool = ctx.enter_context(tc.tile_pool(name="hpool", bufs=3))
        opool = ctx.enter_context(tc.tile_pool(name="opool", bufs=2))
        scr = ctx.enter_context(tc.tile_pool(name="scr", bufs=2))
        psum_x = ctx.enter_context(tc.tile_pool(name="psum_x", bufs=2, space="PSUM"))
        psum_g = ctx.enter_context(tc.tile_pool(name="psum_g", bufs=2, space="PSUM"))
        psum_s = ctx.enter_context(tc.tile_pool(name="psum_s", bufs=2, space="PSUM"))
        psum_o = ctx.enter_context(tc.tile_pool(name="psum_o", bufs=2, space="PSUM"))

        # ---- constants ----
        w_in_sb = consts.tile([128, ND, K2], F16)
        nc.sync.dma_start(out=w_in_sb, in_=w_in_d[:, :, :])
        wsum_sb = consts.tile([128, K2], F16)
        nc.sync.dma_start(out=wsum_sb, in_=wsum_d[:, :])
        ones64_sb = consts.tile([128, K2], F16)
        nc.sync.dma_start(out=ones64_sb, in_=ones64_d[:, :])
        b_t_sb = consts.tile([K2, 1], F32)
        nc.sync.dma_start(out=b_t_sb, in_=b_t_d[:, :])
        w_out_sb = consts.tile([HN + 1, D], F16)
        nc.sync.dma_start(out=w_out_sb, in_=w_out_d[:, :])
        ident = consts.tile([128, 128], F16)
        nc.sync.dma_start(out=ident, in_=ident_d[:, :])
        magic = consts.tile([128, NB], I32)
        nc.gpsimd.memset(magic, RSQRT_MAGIC)
        c15 = consts.tile([128, NB], F32)
        nc.gpsimd.memset(c15, 1.5)
        mhalf = consts.tile([128, NB], F32)
        nc.gpsimd.memset(mhalf, -0.5)
        ceps = consts.tile([128, NB], F32)
        nc.gpsimd.memset(ceps, LN_EPS)

        inv_d = 1.0 / D

        def emit_load(sc):
            s0 = sc * SC
            x_t = xpool.tile([128, NB, D], F16, tag="x_t")
            nc.sync.dma_start(
                out=x_t,
                in_=x_in[s0 : s0 + SC, :].rearrange("(c p) d -> p c d", p=128),
            )
            return x_t

        def emit_stats(x_t):
            """Return rm8 [128, 8] fp16: cols 0:4 = r per block, 4:8 = -mu."""
            if cfg["stats"] == "bn":
                import os
                statsf32 = os.environ.get("KV2_STATSF32", "") == "1"
                mv = stat.tile([128, NB, 2], F32, tag="mv")
                for c in range(NB):
                    src = x_t[:, c, :]
                    if statsf32:
                        xf = scr.tile([128, D], F32, tag="xf32")
                        nc.vector.tensor_copy(out=xf, in_=x_t[:, c, :])
                        src = xf
                    bs = stat.tile([128, 2, nc.vector.BN_STATS_DIM], F32, tag="bs")
                    nc.vector.bn_stats(out=bs[:, 0, :], in_=src[:, 0:512])
                    nc.vector.bn_stats(out=bs[:, 1, :], in_=src[:, 512:1024])
                    nc.vector.bn_aggr(out=mv[:, c, :], in_=bs)
                mu4 = mv[:, :, 0]
                var4 = stat.tile([128, NB], F32, tag="var4")
                nc.gpsimd.tensor_tensor(out=var4, in0=mv[:, :, 1], in1=ceps, op=OP.add)
            else:
                raise ValueError("only stats='bn' is supported on this HW")

            # r = rsqrt(var) via bit-hack + 2 Newton steps (Pool; shift on DVE)
            rm8 = stat.tile([128, 2 * NB], F16, tag="rm8")
            r4 = stat.tile([128, NB], F32, tag="r4")
            t4 = stat.tile([128, NB], F32, tag="t4")
            nc.vector.tensor_scalar(
                out=t4.bitcast(I32), in0=var4.bitcast(I32), scalar1=1, scalar2=None,
                op0=OP.logical_shift_right,
            )
            nc.gpsimd.tensor_tensor(
                out=r4.bitcast(I32), in0=magic, in1=t4.bitcast(I32), op=OP.subtract
            )
            for it in range(2):
                nc.gpsimd.tensor_tensor(out=t4, in0=r4, in1=r4, op=OP.mult)
                nc.gpsimd.tensor_tensor(out=t4, in0=t4, in1=var4, op=OP.mult)
                nc.gpsimd.tensor_tensor(out=t4, in0=t4, in1=mhalf, op=OP.mult)
                nc.gpsimd.tensor_tensor(out=t4, in0=t4, in1=c15, op=OP.add)
                nc.gpsimd.tensor_tensor(out=r4, in0=r4, in1=t4, op=OP.mult)
            # pack into rm_pad [128, 4, 64] fp16: r_c at [:, c, 0], -mu_c at
            # [:, c, 32], zeros elsewhere (so the M=64 transposes produce
            # zero-padded rows; the K=32 rank-1 matmuls then see zeros x
            # zero-weights instead of PSUM garbage).
            rm_pad = stat.tile([128, NB, 64], F16, tag="rmp")
            nc.gpsimd.memset(rm_pad.bitcast(I32), 0)
            nc.vector.tensor_copy(out=rm_pad[:, :, 0:1], in_=r4)
            nc.vector.tensor_scalar(
                out=rm_pad[:, :, 32:33], in0=mu4,
                scalar1=-1.0, scalar2=None, op0=OP.mult,
            )
            return rm_pad

        def emit_front(sc, x_t, rm8):

            # row-land stats: one M=64 transpose per block of the zero-padded
            # [128, 64] column group -> rmT[:, c, :]: row 0 = r_c, row 32 =
            # -mu_c, other rows zero. K=32 rank-1 matmuls read at base 0/32.
            rmT_ps = psum_s.tile([64, NB, 128], F16, tag="rmT")
            for c in range(NB):
                nc.tensor.transpose(rmT_ps[:, c, :], rm8[:, c, :], ident)
            rmT = stat.tile([64, NB, 128], F16, tag="rmTs")
            nc.scalar.copy(out=rmT, in_=rmT_ps)

            # transpose x blocks -> xT fp16 (PSUM), copy out to SBUF
            yt = ytpool.tile([128, ND, SC], F16, tag="yt")
            for c in range(NB):
                xt_ps = psum_x.tile([128, ND, 128], F16, tag="xt")
                for i in range(ND):
                    nc.tensor.transpose(
                        xt_ps[:, i, :], x_t[:, c, i * 128 : (i + 1) * 128], ident
                    )
                eng = nc.vector if cfg["p2"][c] == "dve" else nc.scalar
                if cfg["p2"][c] == "dve":
                    nc.vector.tensor_copy(
                        out=yt[:, :, c * 128 : (c + 1) * 128], in_=xt_ps
                    )
                else:
                    nc.scalar.copy(
                        out=yt[:, :, c * 128 : (c + 1) * 128], in_=xt_ps
                    )

            # in-proj G[0:64] + r_bcast[64:128] in one PSUM bank.
            # The first in-proj matmul owns start=True over all 512 columns;
            # mean corrections accumulate after it (start=False). A start=True
            # placed mid-group corrupts has_written for the other column
            # blocks (HW-observed), so the r_bcast groups run last, when
            # nothing accumulates onto G anymore.
            g_ps = psum_g.tile([128, SC], F32, tag="g")
            for i in range(ND):
                nc.tensor.matmul(
                    g_ps[0:K2, :],
                    lhsT=w_in_sb[:, i, :],
                    rhs=yt[:, i, :],
                    start=(i == 0),
                    stop=False,
                    skip_group_check=True,
                )
            for c in range(NB):
                # mean correction: G[:, c-block] += Wsum x (-mu_c)
                # (-mu_c lives at rmT row 32; Wsum replica at partition 32)
                nc.tensor.matmul(
                    g_ps[0:K2, c * 128 : (c + 1) * 128],
                    lhsT=wsum_sb[32:64, :],
                    rhs=rmT[32:64, c, :],
                    start=False,
                    stop=(c == NB - 1),
                    skip_group_check=True,
                )
            for c in range(NB):
                # r_bcast rows: [64:128, c-block] = ones64 x r_c (row 0)
                nc.tensor.matmul(
                    g_ps[K2 : 2 * K2, c * 128 : (c + 1) * 128],
                    lhsT=ones64_sb[0:32, :],
                    rhs=rmT[0:32, c, :],
                    start=True,
                    stop=True,
                    skip_group_check=True,
                )
            rb_sb = abpool.tile([K2, SC], F16, tag="rb")
            nc.scalar.copy(out=rb_sb, in_=g_ps[K2 : 2 * K2, :])
            logits = abpool.tile([K2, SC], F16, tag="logits")
            nc.vector.tensor_tensor(
                out=logits, in0=g_ps[0:K2, :], in1=rb_sb, op=OP.mult
            )

            alpha_t = abpool.tile([HN, SC], F16, tag="alpha")
            nc.scalar.activation(
                out=alpha_t, in_=logits[0:HN, :], func=AF.Sigmoid,
                bias=b_t_sb[0:HN], scale=1.0,
            )
            bv_t = abpool.tile([HN, SC], F16, tag="bv")
            nc.scalar.activation(
                out=bv_t, in_=logits[HN:K2, :], func=AF.Identity,
                bias=b_t_sb[HN:K2], scale=1.0,
            )
            return alpha_t, bv_t

        def emit_back(sc, x_t, alpha_t, bv_t, h_prev):
            s0 = sc * SC
            # recurrence
            h_t = hpool.tile([HN + 1, SC], F16, tag="h")
            # two packed fp16 1.0s per int32 (fp16 memset untested on HW)
            nc.gpsimd.memset(h_t[HN : HN + 1, :].bitcast(I32), 0x3C003C00)
            nc.vector.tensor_tensor_scan(
                out=h_t[0:HN, :],
                data0=alpha_t,
                data1=bv_t,
                initial=0.0 if h_prev is None else h_prev[0:HN, SC - 1 : SC],
                op0=OP.mult,
                op1=OP.add,
            )

            # out-proj + residual (1-bank halves so psum_o stays at 2 banks)
            o_sb = opool.tile([128, NB, D], F16, tag="o_sb")
            for c in range(NB):
                lhs = h_t[:, c * 128 : (c + 1) * 128]
                for half in range(2):
                    hs = slice(half * 512, (half + 1) * 512)
                    o_ps = psum_o.tile([128, 512], F32, tag="ops")
                    nc.tensor.matmul(
                        o_ps,
                        lhsT=lhs,
                        rhs=w_out_sb[:, hs],
                        start=True,
                        stop=True,
                    )
                    if cfg["p3"][c] == "dve":
                        nc.vector.tensor_tensor(
                            out=o_sb[:, c, hs], in0=o_ps, in1=x_t[:, c, hs],
                            op=OP.add,
                        )
                    else:
                        t = scr.tile([128, 512], F16, tag="p3t")
                        nc.scalar.copy(out=t, in_=o_ps)
                        nc.gpsimd.tensor_tensor(
                            out=o_sb[:, c, hs], in0=t, in1=x_t[:, c, hs],
                            op=OP.add,
                        )
            nc.scalar.dma_start(
                out=out_d[s0 : s0 + SC, :].rearrange("(c p) d -> p c d", p=128),
                in_=o_sb,
            )
            return h_t

        for _rep in range(repeat):
            # 3-stage software pipeline: load(p) | stats(p-1) | main(p-2).
            # (A 4th stage splitting front/back across superchunks hangs the
            # device -- resource cycle at this pool depth; revisit with
            # larger pool bufs if tuning further.)
            xs, rs = {}, {}
            h_prev = None
            for p in range(NSC + 2):
                if p < NSC:
                    xs[p] = emit_load(p)
                if 1 <= p <= NSC:
                    rs[p - 1] = emit_stats(xs[p - 1])
                if p >= 2:
                    sc = p - 2
                    alpha_t, bv_t = emit_front(sc, xs[sc], rs.pop(sc))
                    h_prev = emit_back(sc, xs[sc], alpha_t, bv_t, h_prev)
                    del xs[sc]

    nc.compile()
    return nc


def _prep_host_inputs(x, W_a, b_a, W_in, b_in, W_out, b_out, ln_gamma, ln_beta):
    f = np.float32
    W_cat = np.concatenate(
        [W_a * ln_gamma[None, :], W_in * ln_gamma[None, :]], axis=0
    ).astype(f)  # [64, 1024]
    w_in_host = (
        np.ascontiguousarray(W_cat.T.reshape(ND, 128, K2).transpose(1, 0, 2))
        .astype(np.float16)
    )  # [128, 8, 64]
    wsum_host = np.zeros((128, K2), dtype=np.float16)
    wsum_host[0::32, :] = W_cat.sum(axis=1)[None, :].astype(np.float16)
    ones64_host = np.zeros((128, K2), dtype=np.float16)
    ones64_host[0::32, :] = 1.0
    b_t_host = np.concatenate(
        [b_a + W_a @ ln_beta, b_in + W_in @ ln_beta], axis=0
    ).astype(f)[:, None]  # [64, 1]
    w_out_host = (
        np.ascontiguousarray(np.concatenate([W_out.T, b_out[None, :]], axis=0))
        .astype(np.float16)
    )  # [33, 1024]
    ident_host = np.eye(128, dtype=np.float16)
    shared = {
        "w_in": w_in_host,
        "wsum": wsum_host,
        "ones64": ones64_host,
        "b_t": b_t_host,
        "w_out": w_out_host,
        "ident": ident_host,
    }
    in_maps = [
        {"x": np.ascontiguousarray(x[i]).astype(np.float16), **shared}
        for i in range(B)
    ]
    return in_maps


def run(inputs, trace=False, cfg=CFG):
    key = str(sorted(cfg.items()))
    if key not in _PROGRAM_CACHE:
        _PROGRAM_CACHE[key] = build_program(cfg)
    nc = _PROGRAM_CACHE[key]
    in_maps = _prep_host_inputs(**inputs)
    res = run_bass_kernel_spmd(nc, in_maps, list(range(B)), trace=trace)
    out = np.stack(
        [res.results[i]["out"].astype(np.float32) for i in range(B)], axis=0
    )
    return out, res


def kernel(**inputs):
    out, _ = run(inputs)
    return out



# revision 5
# speedup vs baseline: 1.5504x; 1.5504x over previous
"""DiagonalSSMLayer Trainium2 kernel, v2: fp16 I/O + LN folded into matmul.

Full (unsharded) inputs in, full output out. Data-parallel over batch across
8 NeuronCores (B=8, one batch element per core). Host casts x to fp16
(rel-err budget 2e-2 >> fp16's 5e-4); device loads/stores fp16 (halves HBM
traffic vs f32 -> ~90us DMA floor/core).

Per-core math for x [S=8192, D=1024]:
    mu, var per row (LN stats)
    logits = r * (W_cat @ xT - mu x Wsum) + b      (r = rsqrt(var+eps))
    alpha = sigmoid(logits[0:32]); b = logits[32:64]
    h_t = alpha_t * h_{t-1} + b_t                  (scan along seq)
    out = x + [h;1].T @ [W_out.T; b_out]

Device structure per 512-seq superchunk (16 of them):
  - load x [128, 4, 1024] fp16 (sync HWDGE ring)
  - LN stats: DVE tensor_scalar+accum (S1) + tensor_tensor_reduce (sumsq)
    or bn_stats, per cfg; Pool computes r=rsqrt(var+eps) via Newton + -mu,
    packs into rm8 fp16 [128, 8]
  - PE transposes rm8 -> rmT [8, 128] (row-land stats) and x blocks ->
    xT fp16 in PSUM (is_transpose keeps fp16 => 2x-rate DVE copies out)
  - in-proj: G[64, 512] = W_cat @ xT (PSUM accum) + rank-1 mean correction
    (Wsum x (-mu)) via K=1 matmuls; r_bcast[64, 512] = ones x r via K=1
    matmuls into the same PSUM bank's lower partitions
  - logits = G * r_bcast (DVE); alpha = ACT sigmoid(+b_a); bv = ACT id(+b_in)
  - DVE tensor_tensor_scan -> h fp16, chained via previous chunk's last col
  - out-proj per block: O[128, 1024] f32 = [h;1].T @ [W_out.T; b_out]
  - residual o = O + x -> fp16: DVE fused TT or ACT copy + Pool add (cfg)
  - store o [128, 4, 1024] fp16 (scalar HWDGE ring)
"""

import sys
from contextlib import ExitStack

if "/opt/trn_rl_repo" not in sys.path:
    sys.path.insert(0, "/opt/trn_rl_repo")

import numpy as np

import concourse.bass as bass
import concourse.bacc as bacc
import concourse.tile as tile
from concourse import mybir
from concourse.bass_utils import run_bass_kernel_spmd

F32 = mybir.dt.float32
F16 = mybir.dt.float16
I32 = mybir.dt.int32
OP = mybir.AluOpType
AF = mybir.ActivationFunctionType

B, S, D = 8, 8192, 1024
HN = 32          # H * n state channels
K2 = 2 * HN      # alpha + b fused projection output channels
LN_EPS = 1e-5
RSQRT_MAGIC = 0x5F3759DF

SC = 512         # seq superchunk
NSC = S // SC    # 16
NB = SC // 128   # 4 seq blocks of 128 per superchunk
ND = D // 128    # 8 d-slices

_PROGRAM_CACHE = {}

# engine assignment config
CFG = dict(
    stats="bn",               # "bn": bn_stats/bn_aggr on DVE (the accum_out
                              # variants crash walrus or the device)
    p2=("act", "dve", "act", "dve"),   # xT PSUM->SBUF copy engine per block
    p3="pe",                  # residual: "pe" ident-matmul accum | legacy tuple
    pipe4=True,               # split front/back into separate pipeline stages
    store_eng="gpsimd",       # store DMA queue (keeps ACT stream unblocked)
)


def build_program(cfg=CFG, repeat=1):
    nc = bacc.Bacc("TRN2", target_bir_lowering=False, debug=False, num_devices=B)

    x_in = nc.declare_dram_parameter("x", [S, D], F16, isOutput=False)
    w_in_d = nc.declare_dram_parameter("w_in", [128, ND, K2], F16, isOutput=False)
    # Wsum / ones replicated at partitions 0/32/64/96 so per-block K=1
    # rank-1 matmuls can read lhsT and rhs at the same base partition.
    wsum_d = nc.declare_dram_parameter("wsum", [128, K2], F16, isOutput=False)
    ones64_d = nc.declare_dram_parameter("ones64", [128, K2], F16, isOutput=False)
    b_t_d = nc.declare_dram_parameter("b_t", [K2, 1], F32, isOutput=False)
    w_out_d = nc.declare_dram_parameter("w_out", [HN + 1, D], F16, isOutput=False)
    ident_d = nc.declare_dram_parameter("ident", [128, 128], F16, isOutput=False)
    out_d = nc.declare_dram_parameter("out", [S, D], F16, isOutput=True)

    with tile.TileContext(nc) as tc, ExitStack() as ctx:
        pipe4 = bool(cfg.get("pipe4"))
        consts = ctx.enter_context(tc.tile_pool(name="consts", bufs=1))
        xpool = ctx.enter_context(tc.tile_pool(name="xpool", bufs=5 if pipe4 else 3))
        ytpool = ctx.enter_context(tc.tile_pool(name="ytpool", bufs=2))
        stat = ctx.enter_context(tc.tile_pool(name="stat", bufs=3))
        abpool = ctx.enter_context(tc.tile_pool(name="abpool", bufs=3 if pipe4 else 2))
        hpool = ctx.enter_context(tc.tile_pool(name="hpool", bufs=3))
        opool = ctx.enter_context(tc.tile_pool(name="opool", bufs=2))
        scr = ctx.enter_context(tc.tile_pool(name="scr", bufs=2))
        psum_x = ctx.enter_context(tc.tile_pool(name="psum_x", bufs=2, space="PSUM"))
        psum_g = ctx.enter_context(tc.tile_pool(name="psum_g", bufs=2, space="PSUM"))
        psum_s = ctx.enter_context(tc.tile_pool(name="psum_s", bufs=2, space="PSUM"))
        psum_o = ctx.enter_context(tc.tile_pool(name="psum_o", bufs=2, space="PSUM"))

        # ---- constants ----
        w_in_sb = consts.tile([128, ND, K2], F16)
        nc.sync.dma_start(out=w_in_sb, in_=w_in_d[:, :, :])
        wsum_sb = consts.tile([128, K2], F16)
        nc.sync.dma_start(out=wsum_sb, in_=wsum_d[:, :])
        ones64_sb = consts.tile([128, K2], F16)
        nc.sync.dma_start(out=ones64_sb, in_=ones64_d[:, :])
        b_t_sb = consts.tile([K2, 1], F32)
        nc.sync.dma_start(out=b_t_sb, in_=b_t_d[:, :])
        w_out_sb = consts.tile([HN + 1, D], F16)
        nc.sync.dma_start(out=w_out_sb, in_=w_out_d[:, :])
        ident = consts.tile([128, 128], F16)
        nc.sync.dma_start(out=ident, in_=ident_d[:, :])
        magic = consts.tile([128, NB], I32)
        nc.gpsimd.memset(magic, RSQRT_MAGIC)
        c15 = consts.tile([128, NB], F32)
        nc.gpsimd.memset(c15, 1.5)
        mhalf = consts.tile([128, NB], F32)
        nc.gpsimd.memset(mhalf, -0.5)
        ceps = consts.tile([128, NB], F32)
        nc.gpsimd.memset(ceps, LN_EPS)

        inv_d = 1.0 / D

        def emit_load(sc):
            s0 = sc * SC
            x_t = xpool.tile([128, NB, D], F16, tag="x_t")
            nc.sync.dma_start(
                out=x_t,
                in_=x_in[s0 : s0 + SC, :].rearrange("(c p) d -> p c d", p=128),
            )
            return x_t

        def emit_stats(x_t):
            """Return rm8 [128, 8] fp16: cols 0:4 = r per block, 4:8 = -mu."""
            if cfg["stats"] == "bn":
                import os
                statsf32 = os.environ.get("KV2_STATSF32", "") == "1"
                mv = stat.tile([128, NB, 2], F32, tag="mv")
                for c in range(NB):
                    src = x_t[:, c, :]
                    if statsf32:
                        xf = scr.tile([128, D], F32, tag="xf32")
                        nc.vector.tensor_copy(out=xf, in_=x_t[:, c, :])
                        src = xf
                    bs = stat.tile([128, 2, nc.vector.BN_STATS_DIM], F32, tag="bs")
                    nc.vector.bn_stats(out=bs[:, 0, :], in_=src[:, 0:512])
                    nc.vector.bn_stats(out=bs[:, 1, :], in_=src[:, 512:1024])
                    nc.vector.bn_aggr(out=mv[:, c, :], in_=bs)
                mu4 = mv[:, :, 0]
                var4 = stat.tile([128, NB], F32, tag="var4")
                nc.gpsimd.tensor_tensor(out=var4, in0=mv[:, :, 1], in1=ceps, op=OP.add)
            else:
                raise ValueError("only stats='bn' is supported on this HW")

            # r = rsqrt(var) via bit-hack + 2 Newton steps (Pool; shift on DVE)
            rm8 = stat.tile([128, 2 * NB], F16, tag="rm8")
            r4 = stat.tile([128, NB], F32, tag="r4")
            t4 = stat.tile([128, NB], F32, tag="t4")
            nc.vector.tensor_scalar(
                out=t4.bitcast(I32), in0=var4.bitcast(I32), scalar1=1, scalar2=None,
                op0=OP.logical_shift_right,
            )
            nc.gpsimd.tensor_tensor(
                out=r4.bitcast(I32), in0=magic, in1=t4.bitcast(I32), op=OP.subtract
            )
            for it in range(2):
                nc.gpsimd.tensor_tensor(out=t4, in0=r4, in1=r4, op=OP.mult)
                nc.gpsimd.tensor_tensor(out=t4, in0=t4, in1=var4, op=OP.mult)
                nc.gpsimd.tensor_tensor(out=t4, in0=t4, in1=mhalf, op=OP.mult)
                nc.gpsimd.tensor_tensor(out=t4, in0=t4, in1=c15, op=OP.add)
                nc.gpsimd.tensor_tensor(out=r4, in0=r4, in1=t4, op=OP.mult)
            # pack into rm_pad [128, 4, 64] fp16: r_c at [:, c, 0], -mu_c at
            # [:, c, 32], zeros elsewhere (so the M=64 transposes produce
            # zero-padded rows; the K=32 rank-1 matmuls then see zeros x
            # zero-weights instead of PSUM garbage).
            rm_pad = stat.tile([128, NB, 64], F16, tag="rmp")
            nc.gpsimd.memset(rm_pad.bitcast(I32), 0)
            nc.vector.tensor_copy(out=rm_pad[:, :, 0:1], in_=r4)
            nc.vector.tensor_scalar(
                out=rm_pad[:, :, 32:33], in0=mu4,
                scalar1=-1.0, scalar2=None, op0=OP.mult,
            )
            return rm_pad

        def emit_front(sc, x_t, rm8):

            # row-land stats: one M=64 transpose per block of the zero-padded
            # [128, 64] column group -> rmT[:, c, :]: row 0 = r_c, row 32 =
            # -mu_c, other rows zero. K=32 rank-1 matmuls read at base 0/32.
            rmT_ps = psum_s.tile([64, NB, 128], F16, tag="rmT")
            for c in range(NB):
                nc.tensor.transpose(rmT_ps[:, c, :], rm8[:, c, :], ident)
            rmT = stat.tile([64, NB, 128], F16, tag="rmTs")
            nc.scalar.copy(out=rmT, in_=rmT_ps)

            # transpose x blocks -> xT fp16 (PSUM), copy out to SBUF
            yt = ytpool.tile([128, ND, SC], F16, tag="yt")
            for c in range(NB):
                xt_ps = psum_x.tile([128, ND, 128], F16, tag="xt")
                for i in range(ND):
                    nc.tensor.transpose(
                        xt_ps[:, i, :], x_t[:, c, i * 128 : (i + 1) * 128], ident
                    )
                eng = nc.vector if cfg["p2"][c] == "dve" else nc.scalar
                if cfg["p2"][c] == "dve":
                    nc.vector.tensor_copy(
                        out=yt[:, :, c * 128 : (c + 1) * 128], in_=xt_ps
                    )
                else:
                    nc.scalar.copy(
                        out=yt[:, :, c * 128 : (c + 1) * 128], in_=xt_ps
                    )

            # in-proj G[0:64] + r_bcast[64:128] in one PSUM bank.
            # The first in-proj matmul owns start=True over all 512 columns;
            # mean corrections accumulate after it (start=False). A start=True
            # placed mid-group corrupts has_written for the other column
            # blocks (HW-observed), so the r_bcast groups run last, when
            # nothing accumulates onto G anymore.
            g_ps = psum_g.tile([128, SC], F32, tag="g")
            for i in range(ND):
                nc.tensor.matmul(
                    g_ps[0:K2, :],
                    lhsT=w_in_sb[:, i, :],
                    rhs=yt[:, i, :],
                    start=(i == 0),
                    stop=False,
                    skip_group_check=True,
                )
            for c in range(NB):
                # mean correction: G[:, c-block] += Wsum x (-mu_c)
                # (-mu_c lives at rmT row 32; Wsum replica at partition 32)
                nc.tensor.matmul(
                    g_ps[0:K2, c * 128 : (c + 1) * 128],
                    lhsT=wsum_sb[32:64, :],
                    rhs=rmT[32:64, c, :],
                    start=False,
                    stop=(c == NB - 1),
                    skip_group_check=True,
                )
            for c in range(NB):
                # r_bcast rows: [64:128, c-block] = ones64 x r_c (row 0)
                nc.tensor.matmul(
                    g_ps[K2 : 2 * K2, c * 128 : (c + 1) * 128],
                    lhsT=ones64_sb[0:32, :],
                    rhs=rmT[0:32, c, :],
                    start=True,
                    stop=True,
                    skip_group_check=True,
                )
            rb_sb = abpool.tile([K2, SC], F16, tag="rb")
            nc.scalar.copy(out=rb_sb, in_=g_ps[K2 : 2 * K2, :])
            logits = abpool.tile([K2, SC], F16, tag="logits")
            nc.vector.tensor_tensor(
                out=logits, in0=g_ps[0:K2, :], in1=rb_sb, op=OP.mult
            )

            alpha_t = abpool.tile([HN, SC], F16, tag="alpha")
            nc.scalar.activation(
                out=alpha_t, in_=logits[0:HN, :], func=AF.Sigmoid,
                bias=b_t_sb[0:HN], scale=1.0,
            )
            bv_t = abpool.tile([HN, SC], F16, tag="bv")
            nc.scalar.activation(
                out=bv_t, in_=logits[HN:K2, :], func=AF.Identity,
                bias=b_t_sb[HN:K2], scale=1.0,
            )
            return alpha_t, bv_t

        def emit_back(sc, x_t, alpha_t, bv_t, h_prev):
            s0 = sc * SC
            # recurrence
            h_t = hpool.tile([HN + 1, SC], F16, tag="h")
            # two packed fp16 1.0s per int32 (fp16 memset untested on HW)
            nc.gpsimd.memset(h_t[HN : HN + 1, :].bitcast(I32), 0x3C003C00)
            nc.vector.tensor_tensor_scan(
                out=h_t[0:HN, :],
                data0=alpha_t,
                data1=bv_t,
                initial=0.0 if h_prev is None else h_prev[0:HN, SC - 1 : SC],
                op0=OP.mult,
                op1=OP.add,
            )

            # out-proj + residual (1-bank halves so psum_o stays at 2 banks)
            o_sb = opool.tile([128, NB, D], F16, tag="o_sb")
            for c in range(NB):
                lhs = h_t[:, c * 128 : (c + 1) * 128]
                for half in range(2):
                    hs = slice(half * 512, (half + 1) * 512)
                    o_ps = psum_o.tile([128, 512], F32, tag="ops")
                    if cfg["p3"] == "pe":
                        # residual folded into the PSUM group: += I.T @ x
                        nc.tensor.matmul(
                            o_ps, lhsT=lhs, rhs=w_out_sb[:, hs],
                            start=True, stop=False, skip_group_check=True,
                        )
                        nc.tensor.matmul(
                            o_ps, lhsT=ident, rhs=x_t[:, c, hs],
                            start=False, stop=True, skip_group_check=True,
                        )
                        if (c * 2 + half) % 2 == 0:
                            nc.scalar.copy(out=o_sb[:, c, hs], in_=o_ps)
                        else:
                            nc.vector.tensor_copy(out=o_sb[:, c, hs], in_=o_ps)
                        continue
                    nc.tensor.matmul(
                        o_ps,
                        lhsT=lhs,
                        rhs=w_out_sb[:, hs],
                        start=True,
                        stop=True,
                    )
                    if cfg["p3"][c] == "dve":
                        nc.vector.tensor_tensor(
                            out=o_sb[:, c, hs], in0=o_ps, in1=x_t[:, c, hs],
                            op=OP.add,
                        )
                    else:
                        t = scr.tile([128, 512], F16, tag="p3t")
                        nc.scalar.copy(out=t, in_=o_ps)
                        nc.gpsimd.tensor_tensor(
                            out=o_sb[:, c, hs], in0=t, in1=x_t[:, c, hs],
                            op=OP.add,
                        )
            store_eng = getattr(nc, cfg.get("store_eng", "scalar"))
            store_eng.dma_start(
                out=out_d[s0 : s0 + SC, :].rearrange("(c p) d -> p c d", p=128),
                in_=o_sb,
            )
            return h_t

        for _rep in range(repeat):
            if cfg.get("pipe4"):
                # 4-stage pipeline: load(p) | stats(p-1) | front(p-2) |
                # back(p-3). front(sc+1) overlaps back(sc), breaking the
                # per-superchunk serial chain through all 5 engines.
                xs, rs, fr = {}, {}, {}
                h_prev = None
                for p in range(NSC + 3):
                    if p < NSC:
                        xs[p] = emit_load(p)
                    if 1 <= p <= NSC:
                        rs[p - 1] = emit_stats(xs[p - 1])
                    if 2 <= p <= NSC + 1:
                        sc = p - 2
                        fr[sc] = emit_front(sc, xs[sc], rs.pop(sc))
                    if p >= 3:
                        sc = p - 3
                        alpha_t, bv_t = fr.pop(sc)
                        h_prev = emit_back(sc, xs[sc], alpha_t, bv_t, h_prev)
                        del xs[sc]
            else:
                # 3-stage software pipeline: load(p) | stats(p-1) | main(p-2).
                xs, rs = {}, {}
                h_prev = None
                for p in range(NSC + 2):
                    if p < NSC:
                        xs[p] = emit_load(p)
                    if 1 <= p <= NSC:
                        rs[p - 1] = emit_stats(xs[p - 1])
                    if p >= 2:
                        sc = p - 2
                        alpha_t, bv_t = emit_front(sc, xs[sc], rs.pop(sc))
                        h_prev = emit_back(sc, xs[sc], alpha_t, bv_t, h_prev)
                        del xs[sc]

    nc.compile()
    return nc


def _prep_host_inputs(x, W_a, b_a, W_in, b_in, W_out, b_out, ln_gamma, ln_beta):
    f = np.float32
    W_cat = np.concatenate(
        [W_a * ln_gamma[None, :], W_in * ln_gamma[None, :]], axis=0
    ).astype(f)  # [64, 1024]
    w_in_host = (
        np.ascontiguousarray(W_cat.T.reshape(ND, 128, K2).transpose(1, 0, 2))
        .astype(np.float16)
    )  # [128, 8, 64]
    wsum_host = np.zeros((128, K2), dtype=np.float16)
    wsum_host[0::32, :] = W_cat.sum(axis=1)[None, :].astype(np.float16)
    ones64_host = np.zeros((128, K2), dtype=np.float16)
    ones64_host[0::32, :] = 1.0
    b_t_host = np.concatenate(
        [b_a + W_a @ ln_beta, b_in + W_in @ ln_beta], axis=0
    ).astype(f)[:, None]  # [64, 1]
    w_out_host = (
        np.ascontiguousarray(np.concatenate([W_out.T, b_out[None, :]], axis=0))
        .astype(np.float16)
    )  # [33, 1024]
    ident_host = np.eye(128, dtype=np.float16)
    shared = {
        "w_in": w_in_host,
        "wsum": wsum_host,
        "ones64": ones64_host,
        "b_t": b_t_host,
        "w_out": w_out_host,
        "ident": ident_host,
    }
    in_maps = [
        {"x": np.ascontiguousarray(x[i]).astype(np.float16), **shared}
        for i in range(B)
    ]
    return in_maps


def run(inputs, trace=False, cfg=CFG):
    key = str(sorted(cfg.items()))
    if key not in _PROGRAM_CACHE:
        _PROGRAM_CACHE[key] = build_program(cfg)
    nc = _PROGRAM_CACHE[key]
    in_maps = _prep_host_inputs(**inputs)
    res = run_bass_kernel_spmd(nc, in_maps, list(range(B)), trace=trace)
    out = np.stack(
        [res.results[i]["out"].astype(np.float32) for i in range(B)], axis=0
    )
    return out, res


def kernel(**inputs):
    out, _ = run(inputs)
    return out

